# revision 17
# baseline (speedup 1.0000x reference)
"""Multi-head attention (B=2, S=2048, D=1024, H=16, Dh=64, causal) on 8 TRN2 cores.

Sharding: (batch, head-group) across 8 cores -> core c handles batch c//4 and
heads [4*(c%4), 4*(c%4)+4). Wq/Wk/Wv column-sharded by head group.

Per-core kernel (all matmuls in float32r = full-accuracy fast PE mode):
  - inputs: xT [D,S] (host-transposed x), W slices [D,256]
  - v computed first in natural [S, dh] layout (starts as soon as the first
    xT chunk lands), augmented with a DMA'd ones column
  - qT/kT = W.T @ x.T computed directly in [head*dh, S] layout
  - scoresT tiles [sk=128, sq<=512] = kT.T @ qT per head; causal tiles
    fully above the diagonal are skipped, diagonal-band tiles are trimmed to
    the columns that can be nonzero and masked by precomputed 0/1 masks
  - pT = exp(scoresT/8) (no max subtraction needed; scores ~ N(0,1))
  - ctxT_aug [65, sq] += v_aug.T @ pT accumulated over sk chunks; row 64 is
    the softmax normalizer (ones-column trick)
  - normalize via DVE reciprocal + gpsimd partition_broadcast + DVE multiply
  - output octxT [4, 64, S]; host transposes back to [S, 256]
"""
import sys

if "/opt/trn_rl_repo" not in sys.path:
    sys.path.insert(0, "/opt/trn_rl_repo")

import numpy as np

import concourse.bacc as bacc
import concourse.mybir as mybir
import concourse.tile as tile
from concourse.bass_utils import run_bass_kernel_spmd

F32 = mybir.dt.float32
F32R = mybir.dt.float32r

P = 128          # partitions
S = 2048         # sequence length
D = 1024         # model dim
C = 256          # W columns per core (4 heads x 64)
DH = 64          # head dim
NH = 4           # heads per core
SQT = 512        # sq tile (matmul free dim)
NSQ = S // SQT   # 4
NSK = S // P     # 16
ND = D // P      # 8
N_CORES = 8

_NC_CACHE = {}


def build_nc(loop_n=1):
    key = ("nc", loop_n)
    if key in _NC_CACHE:
        return _NC_CACHE[key]
    nc = bacc.Bacc("TRN2")
    xT = nc.dram_tensor("xT", [D, S], F32R, kind="ExternalInput")
    wq = nc.dram_tensor("wq", [D, C], F32R, kind="ExternalInput")
    wk = nc.dram_tensor("wk", [D, C], F32R, kind="ExternalInput")
    wv = nc.dram_tensor("wv", [D, C], F32R, kind="ExternalInput")
    masks = nc.dram_tensor("masks", [P, 4, SQT], F32R, kind="ExternalInput")
    ones4 = nc.dram_tensor("ones4", [P, NSK * NH], F32R, kind="ExternalInput")
    octxT = nc.dram_tensor("octxT", [NH, DH, S], F32, kind="ExternalOutput")

    import contextlib
    with tile.TileContext(nc) as tc:
        with (tc.For_i(0, loop_n, 1) if loop_n > 1 else contextlib.nullcontext()), \
             tc.tile_pool(name="const", bufs=1) as cp, \
             tc.tile_pool(name="work", bufs=2) as wkp, \
             tc.tile_pool(name="ps", bufs=2, space="PSUM") as ps:
            # ---- persistent SBUF residents ----
            xt = [cp.tile([P, S], F32R, tag=f"xt{k}", name=f"xt{k}") for k in range(ND)]
            wq_sb = cp.tile([P, ND, C], F32R, tag="wq", name="wq_sb")
            wk_sb = cp.tile([P, ND, C], F32R, tag="wk", name="wk_sb")
            wv_sb = cp.tile([P, ND, C], F32R, tag="wv", name="wv_sb")
            mask_sb = cp.tile([P, 4, SQT], F32R, tag="mask", name="mask_sb")
            qT_sb = [cp.tile([P, S], F32R, tag=f"qT{i}", name=f"qT{i}") for i in range(2)]
            kT_sb = [cp.tile([P, S], F32R, tag=f"kT{i}", name=f"kT{i}") for i in range(2)]
            va = cp.tile([P, NSK, NH, DH + 1], F32R, tag="va", name="va")

            # ---- input DMAs (order = prefetch priority) ----
            # xt arrives in column quarters so chunk-c work starts early
            wv3 = wv.rearrange("(ko p) c -> ko p c", p=P)
            nc.sync.dma_start(wv_sb[:, 0], wv3[0])
            for k in range(ND):
                nc.sync.dma_start(xt[k][:, 0:SQT], xT[k * P:(k + 1) * P, 0:SQT])
                if k + 1 < ND:
                    nc.sync.dma_start(wv_sb[:, k + 1], wv3[k + 1])
            nc.sync.dma_start(wq_sb[:], wq.rearrange("(ko p) c -> p ko c", p=P))
            nc.sync.dma_start(wk_sb[:], wk.rearrange("(ko p) c -> p ko c", p=P))
            nc.scalar.dma_start(mask_sb[:], masks[:])
            nc.scalar.dma_start(
                va[:, :, :, DH],
                ones4.rearrange("p (j h) -> p j h", j=NSK))
            for q in range(1, NSQ):
                for k in range(ND):
                    nc.sync.dma_start(xt[k][:, q * SQT:(q + 1) * SQT],
                                      xT[k * P:(k + 1) * P, q * SQT:(q + 1) * SQT])

            def emit_proj(c):
                sq = slice(c * SQT, (c + 1) * SQT)
                for j in range(4 * c, 4 * c + 4):
                    psv = ps.tile([P, C], F32, tag="B", bufs=2, name="psv")
                    for k in range(ND):
                        nc.tensor.matmul(psv[:],
                                         xt[k][:, j * P:(j + 1) * P],
                                         wv_sb[:, k],
                                         start=(k == 0), stop=(k == ND - 1))
                    nc.vector.tensor_copy(
                        va[:, j, :, 0:DH],
                        psv[:].rearrange("p (h d) -> p h d", h=NH))
                for hp in range(2):
                    psq = ps.tile([P, SQT], F32, tag="B", bufs=2, name="psq")
                    for k in range(ND):
                        nc.tensor.matmul(psq[:],
                                         wq_sb[:, k, hp * P:(hp + 1) * P],
                                         xt[k][:, sq],
                                         start=(k == 0), stop=(k == ND - 1))
                    nc.vector.tensor_copy(qT_sb[hp][:, sq], psq[:])
                    psk = ps.tile([P, SQT], F32, tag="B", bufs=2, name="psk")
                    for k in range(ND):
                        nc.tensor.matmul(psk[:],
                                         wk_sb[:, k, hp * P:(hp + 1) * P],
                                         xt[k][:, sq],
                                         start=(k == 0), stop=(k == ND - 1))
                    nc.vector.tensor_copy(kT_sb[hp][:, sq], psk[:])

            def emit_attn(c):
                sq = slice(c * SQT, (c + 1) * SQT)
                jmax = 4 * c + 4
                for hp in range(2):
                    # head pair h0 = 2*hp, h1 = 2*hp+1 processed together so
                    # their K=64 QK matmuls sit adjacent (PE row-group overlap)
                    pscs = [ps.tile([DH + 1, SQT], F32, tag="A", bufs=2,
                                    name=f"psc{i}") for i in range(2)]
                    # untrimmed sk chunks (j < 4c) in pairs: two QK outputs in
                    # one 2-bank psum tile, ONE exp op over 1024 columns
                    for jp in range(2 * c):
                        j0 = 2 * jp
                        for i in range(2):
                            off = DH * i
                            pss = ps.tile([P, 2, SQT], F32, tag="S", bufs=2,
                                          name=f"pss{i}")
                            for u in range(2):
                                nc.tensor.matmul(pss[:, u],
                                                 kT_sb[hp][off:off + DH,
                                                           (j0 + u) * P:(j0 + u + 1) * P],
                                                 qT_sb[hp][off:off + DH, sq],
                                                 start=True, stop=True)
                            pt = wkp.tile([P, 2, SQT], F32R, tag="pT", bufs=8,
                                          name=f"pt{i}")
                            nc.scalar.activation(pt[:], pss[:],
                                                 mybir.ActivationFunctionType.Exp,
                                                 scale=0.125)
                            for u in range(2):
                                nc.tensor.matmul(pscs[i][:],
                                                 va[:, j0 + u, 2 * hp + i, :],
                                                 pt[:, u],
                                                 start=(j0 + u == 0), stop=False)
                    # diagonal band: trimmed singles with masking
                    for j in range(4 * c, jmax):
                        t = j - 4 * c
                        lo = P * t
                        w = SQT - lo
                        sqw = slice(c * SQT + lo, (c + 1) * SQT)
                        psss = []
                        for i in range(2):
                            off = DH * i
                            pss = ps.tile([P, 2, SQT], F32, tag="S", bufs=2,
                                          name=f"pss{i}")
                            nc.tensor.matmul(pss[:, 0, 0:w],
                                             kT_sb[hp][off:off + DH, j * P:(j + 1) * P],
                                             qT_sb[hp][off:off + DH, sqw],
                                             start=True, stop=True)
                            psss.append(pss)
                        for i in range(2):
                            pt = wkp.tile([P, 2, SQT], F32R, tag="pT", bufs=8,
                                          name=f"pt{i}")
                            nc.scalar.activation(pt[:, 0, lo:SQT], psss[i][:, 0, 0:w],
                                                 mybir.ActivationFunctionType.Exp,
                                                 scale=0.125)
                            mw = min(P, w)
                            nc.vector.tensor_mul(pt[:, 0, lo:lo + mw],
                                                 pt[:, 0, lo:lo + mw],
                                                 mask_sb[:, t, lo:lo + mw])
                            nc.tensor.matmul(pscs[i][:, lo:SQT],
                                             va[:, j, 2 * hp + i, :],
                                             pt[:, 0, lo:SQT],
                                             start=(j == 0), stop=(j == jmax - 1))
                    for i in range(2):
                        h = 2 * hp + i
                        recip = wkp.tile([1, SQT], F32, tag="recip", bufs=2,
                                         name="recip")
                        nc.vector.reciprocal(recip[:], pscs[i][DH:DH + 1, :])
                        bc = wkp.tile([DH, SQT], F32, tag="bc", bufs=2, name="bc")
                        nc.gpsimd.partition_broadcast(bc[:], recip[:])
                        ctx_sb = wkp.tile([DH, SQT], F32, tag="ctx", bufs=2,
                                          name="ctx_sb")
                        nc.vector.tensor_mul(ctx_sb[:], pscs[i][0:DH, :], bc[:])
                        nc.scalar.dma_start(octxT[h, :, sq], ctx_sb[:])

            for c in range(NSQ):
                emit_proj(c)
                emit_attn(c)
    nc.compile()
    _NC_CACHE[key] = nc
    return nc


def _masks_np():
    # mask_t[p, f] = 1.0 if (128*t + p) <= f else 0  (allowed = key pos <= query pos)
    p = np.arange(P)[:, None, None]
    t = np.arange(4)[None, :, None]
    f = np.arange(SQT)[None, None, :]
    return np.ascontiguousarray(((P * t + p) <= f).astype(np.float32))


def make_in_maps(x, Wq, Wk, Wv):
    x = np.asarray(x, dtype=np.float32)
    Wq = np.asarray(Wq, dtype=np.float32)
    Wk = np.asarray(Wk, dtype=np.float32)
    Wv = np.asarray(Wv, dtype=np.float32)
    masks = _masks_np()
    ones4 = np.ones((P, NSK * NH), np.float32)
    in_maps = []
    for core in range(N_CORES):
        b, g = divmod(core, 4)
        cols = slice(C * g, C * (g + 1))
        in_maps.append({
            "xT": np.ascontiguousarray(x[b].T),
            "wq": np.ascontiguousarray(Wq[:, cols]),
            "wk": np.ascontiguousarray(Wk[:, cols]),
            "wv": np.ascontiguousarray(Wv[:, cols]),
            "masks": masks,
            "ones4": ones4,
        })
    return in_maps


def assemble_out(results):
    out = np.empty((2, S, D), np.float32)
    for core in range(N_CORES):
        b, g = divmod(core, 4)
        octxT = results[core]["octxT"]            # [4, 64, S]
        out[b, :, C * g:C * (g + 1)] = octxT.transpose(2, 0, 1).reshape(S, C)
    return out


def kernel(x, Wq, Wk, Wv):
    nc = build_nc()
    in_maps = make_in_maps(x, Wq, Wk, Wv)
    res = run_bass_kernel_spmd(nc, in_maps, core_ids=list(range(N_CORES)))
    return assemble_out(res.results)
